# revision 4
# baseline (speedup 1.0000x reference)
"""Trainium2 Bass kernel for grouped multi-head attention (nn_Attention_8263517077742).

Reference computation (per batch b, group g, with x [2048, 512]):
  xn   = x / max(||x||_2, eps) * sqrt(512)        (rmsnorm over feature dim)
  q    = (xn * gamma_q) @ wq[g].T                 -> 8 heads of 64
  k,v  = (xn * gamma_c) @ wkv[g].T                -> 8 heads of 64
  null k/v prepended along key sequence; scores masked by mask[b]; softmax;
  merged heads projected by wout[g].

Sharding: 8 cores = 4 (b,g) instances x 2 query-sequence halves. Each core
computes attention for its 1024 queries over the shared key set, so output
slices are disjoint and no cross-core communication is needed.

Key ideas on top of the v1 kernel:
  - masked keys contribute exp(-1e30) = 0 to softmax and AV, so the key set
    is compacted host-side to just the unmasked rows (~half of 2048), padded
    with zero rows + -1e30 bias to a fixed NK = NKT*128. Queries (all rows)
    and keys (unmasked rows) become separate input tensors; the rotation
    trick of v1 is no longer needed.
  - all matmul operands are bf16 (psum accumulation stays fp32): same PE rate
    as float32r but native full-rate on HW, half the SBUF traffic, and 2x DVE
    on pure-bf16 copies. fp32 is kept for the norm math and exp bias.
  - output projection contracts over 128 rows (head pairs merged in one
    partition tile) instead of 64, halving its matmul count.
  - psum->sbuf copies of the x transposes go to DVE, not the scalar engine:
    the Activation engine is the exp bottleneck.
  - scores are computed transposed (sT [nk, nq]) so softmax needs no
    transposes: exp on ScalarE with the key mask as a per-partition bias,
    denominators via an extra ones-column per head in v, normalization via
    reciprocal + gpsimd partition-broadcast + multiply.
  - no max-subtraction in softmax (scores are O(10); fp32 exp cannot overflow)
  - null k/v handled as a rank-1 update closing each AV accumulation.
"""

import sys
from contextlib import ExitStack

import numpy as np
import ml_dtypes

if "/opt/trn_rl_repo" not in sys.path:
    sys.path.insert(0, "/opt/trn_rl_repo")

import concourse.bass as bass  # noqa: E402
import concourse.mybir as mybir  # noqa: E402
from concourse import bacc  # noqa: E402
from concourse.tile import TileContext  # noqa: E402
from concourse.masks import make_identity  # noqa: E402

P = 128
D = 512           # feature dim
E = 512           # inner dim (8 heads x 64)
NQ = 1024         # queries per core
H = 8
DH = 64
ET = E // P       # 4 e-tiles
DT = D // P       # 4 d-tiles
QT = NQ // P      # 8 query tiles
VEXT = H * (DH + 1)   # 520: per-head v columns + ones column
F32 = mybir.dt.float32
BF16 = mybir.dt.bfloat16

B, G = 2, 2
NKT_DEFAULT = 9   # key tiles: 1152 key slots; ~1024+5sigma needed for a random mask


def build_nc(reps=1, nkt=NKT_DEFAULT):
    NK = nkt * P
    nc = bacc.Bacc(
        trn_type="TRN2",
        target_bir_lowering=False,
        debug=False,
        enable_asserts=False,
        num_devices=8,
    )
    xqt_ext = nc.declare_dram_parameter("xq_t", [D, NQ], BF16, isOutput=False)
    xct_ext = nc.declare_dram_parameter("xc_t", [D, NK], BF16, isOutput=False)
    # packed weight walls (single DMA each):
    #   wall1 cols: per dj-block of 1024: [wq_t rows dj | wk_t rows dj]; then 4x8 nks
    #   wall2 cols: per block of 1024: [wv_t rows dj | wo_t rows j]
    w1_ext = nc.declare_dram_parameter("wall1", [P, DT * 2 * E + ET * H], BF16, isOutput=False)
    w2_ext = nc.declare_dram_parameter("wall2", [P, DT * 2 * E], BF16, isOutput=False)
    mb_ext = nc.declare_dram_parameter("maskbias", [P, nkt], F32, isOutput=False)
    nve_ext = nc.declare_dram_parameter("nullv_ext", [1, VEXT], BF16, isOutput=False)
    out_ext = nc.declare_dram_parameter("out", [NQ, D], F32, isOutput=True)

    def fchunks(total, step=512):
        c = []
        o = 0
        while o < total:
            c.append((o, min(step, total - o)))
            o += step
        return c

    with TileContext(nc) as tc, ExitStack() as ctx:
        if reps > 1:
            ctx.enter_context(tc.For_i(
                0, reps, 1,
                hint_engines=(
                    mybir.EngineType.PE, mybir.EngineType.DVE,
                    mybir.EngineType.Activation, mybir.EngineType.SP,
                    mybir.EngineType.Pool,
                ),
            ))
        # ---- pools that live through prologue + main loop ----
        persist = ctx.enter_context(tc.tile_pool(name="persist", bufs=1))
        kT = [persist.tile([P, NK], BF16, name=f"kT{j}", tag=f"kT{j}") for j in range(ET)]
        qT = [persist.tile([P, NQ], BF16, name=f"qT{j}", tag=f"qT{j}") for j in range(ET)]
        v_ext = [persist.tile([P, VEXT], BF16, name=f"vx{i}", tag=f"vx{i}") for i in range(nkt)]
        # merged heads, pairs per partition tile: rows of mg{j} are
        # e = 128*j .. 128*j+127  (head 2j in rows 0..63, head 2j+1 in 64..127)
        mergedT = [persist.tile([P, NQ], BF16, name=f"mg{j}", tag=f"mg{j}") for j in range(ET)]
        mb_sb = persist.tile([P, nkt], F32, name="mb", tag="mb")
        nve_sb = persist.tile([1, VEXT], BF16, name="nve", tag="nve")

        with tc.tile_pool(name="prolog", bufs=1) as prolog, \
             tc.tile_pool(name="ppsum", bufs=2, space="PSUM") as ppsum:
            xnTq = [prolog.tile([P, NQ], BF16, name=f"xnTq{j}", tag=f"xnTq{j}") for j in range(DT)]
            xnTc = [prolog.tile([P, NK], BF16, name=f"xnTc{j}", tag=f"xnTc{j}") for j in range(DT)]
            onesc = prolog.tile([P, H], BF16, name="onesc", tag="onesc")
            nc.vector.memset(onesc[:, :], 1.0)
            # dummy ops: pull the sqrt/exp table-set loads (~1.3us each) off
            # the first norm tile's critical path
            warm = prolog.tile([1, 2], F32, name="warm", tag="warm")
            nc.vector.memset(warm[:, :], 1.0)
            # pull the sqrt table-set load (~1.3us) off the first norm's
            # critical path; the exp set is loaded after the last sqrt (below)
            # so each set loads exactly once
            nc.scalar.activation(warm[0:1, 0:1], warm[0:1, 0:1],
                                 mybir.ActivationFunctionType.Sqrt)
            wall1 = prolog.tile([P, DT * 2 * E + ET * H], BF16, name="wall1", tag="wall1")
            wall2 = prolog.tile([P, DT * 2 * E], BF16, name="wall2", tag="wall2")

            def wq_s(dj, lo, hi):
                return wall1[:, dj * 1024 + lo:dj * 1024 + hi]

            def wk_s(dj, lo, hi):
                return wall1[:, dj * 1024 + E + lo:dj * 1024 + E + hi]

            def nkc_s(j):
                # null-key column for kT[j] (the fixed last key slot NK-1)
                return wall1[:, DT * 2 * E + j:DT * 2 * E + j + 1]

            def wv_s(dj):
                return wall2[:, dj * 1024:dj * 1024 + E]

            def wo_s(j):
                return wall2[:, j * 1024 + E:j * 1024 + 2 * E]
            # per-column rmsnorm scales (alpha = 1/||x||; sqrt(D)*gamma folded
            # into the weights host-side) broadcast across partitions, plus
            # per-partition alpha columns for the v projection
            acbc = prolog.tile([P, NK], F32, name="acbc", tag="acbc")
            acol = [prolog.tile([P, 1], F32, name=f"acol{t}", tag=f"acol{t}")
                    for t in range(nkt)]
            po1_sb = [prolog.tile([P, D], F32, name=f"po1_{cq}", tag=f"po1_{cq}")
                      for cq in range(QT)]

            # raw x arrives pre-transposed (bf16) from the host; the context
            # side comes first (its norm + k projection gate the first scores)
            for dj in range(DT):
                nc.sync.dma_start(out=xnTc[dj][:, :], in_=xct_ext[dj * P:(dj + 1) * P, :])
            for dj in range(DT):
                nc.sync.dma_start(out=xnTq[dj][:, :], in_=xqt_ext[dj * P:(dj + 1) * P, :])
            nc.sync.dma_start(out=wall1[:, :], in_=w1_ext[:, :])
            nc.sync.dma_start(out=wall2[:, :], in_=w2_ext[:, :])
            nc.sync.dma_start(out=mb_sb[:, :], in_=mb_ext[:, :])
            nc.sync.dma_start(out=nve_sb[:, :], in_=nve_ext[:, :])

            # -- prologue-only pools: norms (column sums of x^2 via matmul) --
            with tc.tile_pool(name="sqpool", bufs=1) as sqpool, \
                 tc.tile_pool(name="npsum", bufs=1, space="PSUM") as npsum, \
                 tc.tile_pool(name="tpsum", bufs=1, space="PSUM") as tpsum:
                ident = sqpool.tile([P, P], F32, name="ident", tag="ident")
                make_identity(nc, ident[:, :])
                ones1 = sqpool.tile([P, 1], BF16, name="ones1", tag="ones1")
                nc.vector.memset(ones1[:, :], 1.0)
                aqbc = sqpool.tile([P, NQ], F32, name="aqbc", tag="aqbc")

                def emit_alpha(xT, n, abc_tile, pfx):
                    xsq = [sqpool.tile([P, n], BF16, name=f"xsq{pfx}{dj}", tag=f"xsq{pfx}{dj}")
                           for dj in range(DT)]
                    for dj in range(DT):
                        nc.vector.tensor_mul(xsq[dj][:, 0:n], xT[dj][:, :], xT[dj][:, :])
                    ps = npsum.tile([1, NK], F32, name=f"ps{pfx}", tag="ps")
                    for (o, w) in fchunks(n):
                        for dj in range(DT):
                            nc.tensor.matmul(
                                ps[:, o:o + w],
                                lhsT=ones1[:, :],
                                rhs=xsq[dj][:, o:o + w],
                                start=(dj == 0), stop=(dj == DT - 1),
                            )
                    nrm = sqpool.tile([1, NK], F32, name="nrm", tag="nrm")
                    nc.scalar.activation(nrm[0:1, 0:n], ps[0:1, 0:n],
                                         mybir.ActivationFunctionType.Sqrt)
                    nc.vector.tensor_scalar_max(nrm[0:1, 0:n], nrm[0:1, 0:n], 1e-12)
                    arow = sqpool.tile([1, NK], F32, name="arow", tag="arow")
                    nc.vector.reciprocal_approx_fast(arow[0:1, 0:n], nrm[0:1, 0:n])
                    nc.gpsimd.partition_broadcast(abc_tile[:, 0:n], arow[0:1, 0:n])

                emit_alpha(xnTc, NK, acbc, "c")
                emit_alpha(xnTq, NQ, aqbc, "q")
                nc.scalar.activation(warm[0:1, 1:2], warm[0:1, 1:2],
                                     mybir.ActivationFunctionType.Exp)
                # alpha columns for the v projection: transpose a 128-col block
                # of the broadcast (every column of the transpose equals the
                # per-token alpha column for that key tile)
                for t in range(nkt):
                    tp = tpsum.tile([P, P], F32, name="tp", tag="tp")
                    nc.tensor.transpose(tp[:, :], acbc[:, t * P:(t + 1) * P], ident[:, :])
                    nc.vector.tensor_copy(acol[t][:, :], tp[:, 0:1])

                # q^T projection (alpha_q folded in on the psum->sbuf copy)
                if True:
                    for j in range(ET):
                        for (o, w) in fchunks(NQ):
                            pq = ppsum.tile([P, 512], F32, name="pk", tag="pk")
                            for dj in range(DT):
                                nc.tensor.matmul(
                                    pq[:, 0:w],
                                    lhsT=wq_s(dj, j * P, (j + 1) * P),
                                    rhs=xnTq[dj][:, o:o + w],
                                    start=(dj == 0), stop=(dj == DT - 1),
                                )
                            nc.vector.tensor_mul(qT[j][:, o:o + w], pq[:, 0:w],
                                                 aqbc[:, o:o + w])


            # -- helpers emitted just-in-time inside the attention loop --
            def emit_vproj(i):
                pv = ppsum.tile([P, 512], F32, name="pk", tag="pk")
                for dj in range(DT):
                    nc.tensor.matmul(
                        pv[:, :],
                        lhsT=xnTc[dj][:, i * P:(i + 1) * P],
                        rhs=wv_s(dj),
                        start=(dj == 0), stop=(dj == DT - 1),
                    )
                src = pv[:, :].rearrange("p (a d) -> p a d", a=H)
                dst = v_ext[i][:, :].rearrange("p (a r) -> p a r", a=H)
                nc.vector.tensor_scalar_mul(dst[:, :, 0:DH], src[:, :, :], acol[i][:, :])
                nc.vector.tensor_copy(dst[:, :, DH:DH + 1],
                                      onesc[:, :].rearrange("p (a r) -> p a r", a=H))
                if i == nkt - 1:
                    # fixed null-kv slot NK-32 (partition starts must be
                    # 32-aligned): overwrite that padding row with the null v
                    # (+ its ones column), bypassing alpha
                    nc.vector.tensor_copy(v_ext[i][P - 32:P - 31, :], nve_sb[0:1, :])

            def emit_kproj(j):
                for (o, w) in fchunks(NK):
                    pk = ppsum.tile([P, 512], F32, name="pk", tag="pk")
                    for dj in range(DT):
                        nc.tensor.matmul(
                            pk[:, 0:w],
                            lhsT=wk_s(dj, j * P, (j + 1) * P),
                            rhs=xnTc[dj][:, o:o + w],
                            start=(dj == 0), stop=(dj == DT - 1),
                        )
                    nc.vector.tensor_mul(kT[j][:, o:o + w], pk[:, 0:w], acbc[:, o:o + w])
                nc.vector.tensor_copy(kT[j][:, NK - 32:NK - 31], nkc_s(j))

            # ---- main attention loop (v/k projections interleaved) ----
            with tc.tile_pool(name="sps", bufs=2, space="PSUM") as sps, \
                 tc.tile_pool(name="avps", bufs=1, space="PSUM") as avps, \
                 tc.tile_pool(name="ppool", bufs=3) as ppool, \
                 tc.tile_pool(name="opool", bufs=3) as opool, \
                 tc.tile_pool(name="rpool", bufs=2) as rpool:

                def emit_oproj_half1(cq):
                    # first half of the output projection (head pairs 0 and 1),
                    # staged to SBUF; interleaved into the act-paced head loop
                    pp = ppsum.tile([P, 512], F32, name="pk", tag="pk")
                    for j in range(2):
                        nc.tensor.matmul(
                            pp[:, :],
                            lhsT=mergedT[j][:, cq * P:(cq + 1) * P],
                            rhs=wo_s(j),
                            start=(j == 0), stop=(j == 1),
                        )
                    nc.vector.tensor_copy(po1_sb[cq][:, :], pp[:, :])

                def emit_oproj_q3(cq):
                    # third quarter (head pair 2), accumulated into the staged
                    # SBUF partials during head 7
                    pp = ppsum.tile([P, 512], F32, name="pk", tag="pk")
                    nc.tensor.matmul(
                        pp[:, :],
                        lhsT=mergedT[2][:, cq * P:(cq + 1) * P],
                        rhs=wo_s(2),
                        start=True, stop=True,
                    )
                    nc.vector.tensor_add(po1_sb[cq][:, :], pp[:, :], po1_sb[cq][:, :])

                emit_vproj(0)
                emit_vproj(1)
                emit_kproj(0)
                for h in range(H):
                    j, off = h // 2, DH * (h % 2)
                    av = avps.tile([DH + 1, NQ], F32, name="av", tag="av")
                    # rows 0..63 = v part, row 64 = softmax denominators r
                    for t in range(nkt):
                        if h == 0 and t + 2 < nkt:
                            emit_vproj(t + 2)
                        # k projections and the first output-projection half are
                        # emitted mid-head, a few tiles after the head's first
                        # scores, so they fill PE idle in the act-paced pipeline
                        # without stalling the next exp behind cross-engine deps
                        if t == 3 and h in (1, 3, 5):
                            emit_kproj((h + 1) // 2)
                        if t in (5, 7) and 4 <= h <= 7:
                            emit_oproj_half1(2 * (h - 4) + (t == 7))
                        st = sps.tile([P, NQ], F32, name="st", tag="st")
                        for (o, w) in fchunks(NQ):
                            nc.tensor.matmul(
                                st[:, o:o + w],
                                lhsT=kT[j][off:off + DH, t * P:(t + 1) * P],
                                rhs=qT[j][off:off + DH, o:o + w],
                                start=True, stop=True,
                            )
                        pt = ppool.tile([P, NQ], BF16, name="pt", tag="pt")
                        nc.scalar.activation(
                            pt[:, :], st[:, :], mybir.ActivationFunctionType.Exp,
                            bias=mb_sb[:, t:t + 1], scale=1.0,
                        )
                        for (o, w) in fchunks(NQ):
                            nc.tensor.matmul(
                                av[:, o:o + w],
                                lhsT=v_ext[t][:, h * 65:h * 65 + 65],
                                rhs=pt[:, o:o + w],
                                start=(t == 0), stop=(t == nkt - 1),
                            )
                    # stage av out of PSUM so the next head can reuse the bank;
                    # the last head has no successor, so it skips the copy and
                    # normalizes straight from PSUM (shorter path to final proj)
                    if h < H - 1:
                        avc = rpool.tile([DH + 1, NQ], F32, name="avc", tag="avc")
                        nc.vector.tensor_copy(avc[:, :], av[:, :])
                    else:
                        avc = av
                    # the approx-recip custom op misreads PSUM and nonzero
                    # base-partition inputs: stage the denominator row through
                    # a base-0 SBUF tile first. The last head normalizes in
                    # column halves so the output projection's final quarter
                    # can start on the first half ~2us earlier.
                    for (o, w) in (fchunks(NQ) if h == H - 1 else [(0, NQ)]):
                        denc = rpool.tile([1, NQ], F32, name="denc", tag="denc")
                        nc.vector.tensor_copy(denc[0:1, 0:w], avc[DH:DH + 1, o:o + w])
                        recip = rpool.tile([1, NQ], F32, name="recip", tag="recip", bufs=1)
                        nc.vector.reciprocal_approx_fast(recip[0:1, 0:w], denc[0:1, 0:w])
                        rbc = rpool.tile([DH, NQ], F32, name="rbc", tag="rbc")
                        nc.gpsimd.partition_broadcast(rbc[:, 0:w], recip[0:1, 0:w])
                        nc.vector.tensor_mul(mergedT[j][off:off + DH, o:o + w],
                                             avc[0:DH, o:o + w], rbc[:, 0:w])

                # ---- output projection tail ----
                # head-pair 2 first: it only needs mergedT[2] (done at head 5),
                # so these matmuls run during head 7's normalize chain and keep
                # the PE p-state warm through the would-be idle window
                for cq in range(QT):
                    emit_oproj_q3(cq)
                # final quarter (head pair 3, gated on head 7) + store
                for cq in range(QT):
                    pp = ppsum.tile([P, 512], F32, name="pk", tag="pk")
                    nc.tensor.matmul(
                        pp[:, :],
                        lhsT=mergedT[3][:, cq * P:(cq + 1) * P],
                        rhs=wo_s(3),
                        start=True, stop=True,
                    )
                    osb = opool.tile([P, D], F32, name="osb", tag="osb")
                    nc.vector.tensor_add(osb[:, :], pp[:, :], po1_sb[cq][:, :])
                    nc.sync.dma_start(out=out_ext[cq * P:(cq + 1) * P, :], in_=osb[:, :])

    nc.compile()
    return nc


_NC_CACHE = {}


def get_nc(nkt=NKT_DEFAULT):
    if nkt not in _NC_CACHE:
        _NC_CACHE[nkt] = build_nc(nkt=nkt)
    return _NC_CACHE[nkt]


def make_in_maps(x, mask, gamma_q, gamma_c, wq, wkv, wout, null_kv, nkt=None):
    x = np.asarray(x, dtype=np.float32)
    mask = np.asarray(mask)
    gamma_q = np.asarray(gamma_q, dtype=np.float32)
    gamma_c = np.asarray(gamma_c, dtype=np.float32)
    wq = np.asarray(wq, dtype=np.float32)
    wkv = np.asarray(wkv, dtype=np.float32)
    wout = np.asarray(wout, dtype=np.float32)
    null_kv = np.asarray(null_kv, dtype=np.float32)

    sqD = np.float32(np.sqrt(D))
    scale = np.float32(DH ** -0.5)
    DI = E
    bf = ml_dtypes.bfloat16

    if nkt is None:
        counts = [int(mask[b].sum()) for b in range(B)]
        # the null key lives at slot NK-32 (32-aligned partition), so the
        # compacted keys must fit below it
        nkt = max(NKT_DEFAULT, -(-(max(counts) + 32) // P))
    NK = nkt * P

    per_g = {}
    for g in range(G):
        wq_t = np.ascontiguousarray((wq[g] * (gamma_q[g] * sqD * scale)[None, :]).T).astype(bf)
        wk_t = np.ascontiguousarray((wkv[g][:DI] * (gamma_c[g] * sqD)[None, :]).T).astype(bf)
        wv_t = np.ascontiguousarray((wkv[g][DI:] * (gamma_c[g] * sqD)[None, :]).T).astype(bf)
        wo_t = np.ascontiguousarray(wout[g].T).astype(bf)
        nullk = null_kv[0, g, :, 0, :]            # [H, DH]
        nks = np.zeros((E, H), np.float32)
        for h in range(H):
            nks[h * DH:(h + 1) * DH, h] = nullk[h]
        nve = np.zeros((H, VEXT), np.float32)
        for h in range(H):
            nve[h, h * 65:h * 65 + 64] = null_kv[1, g, h, 0, :]
            nve[h, h * 65 + 64] = 1.0
        nullk_full = nullk.reshape(E)  # e = h*64+dh
        # packed weight walls (see build_nc): per dj block of 1024 cols,
        # wall1 = [wq | wk] + null-k columns tail, wall2 = [wv | wo]
        wall1 = np.zeros((P, DT * 2 * E + ET * H), bf)
        wall2 = np.zeros((P, DT * 2 * E), bf)
        for dj in range(DT):
            rows = slice(dj * P, (dj + 1) * P)
            wall1[:, dj * 1024:dj * 1024 + E] = wq_t[rows]
            wall1[:, dj * 1024 + E:dj * 1024 + 2 * E] = wk_t[rows]
            wall1[:, DT * 2 * E + dj] = nullk_full[rows].astype(bf)
            wall2[:, dj * 1024:dj * 1024 + E] = wv_t[rows]
            wall2[:, dj * 1024 + E:dj * 1024 + 2 * E] = wo_t[rows]
        per_g[g] = (wall1, wall2, nve.astype(bf)[None, :].sum(axis=1))

    per_b = {}
    for b in range(B):
        idx = np.flatnonzero(mask[b])
        cnt = len(idx)
        mb = np.full(NK, np.float32(-1e30), np.float32)
        mb[:cnt] = 0.0
        mb[NK - 32] = 0.0  # null-kv slot
        per_b[b] = (idx, np.ascontiguousarray(mb.reshape(nkt, P).T))

    in_maps = []
    for c in range(8):
        b, g, half = c // 4, (c // 2) % 2, c % 2
        wall1, wall2, nve = per_g[g]
        idx, mb_c = per_b[b]
        xc_t = np.zeros((D, NK), bf)
        xc_t[:, :len(idx)] = x[b, g][idx].T.astype(bf)
        xq_t = np.ascontiguousarray(x[b, g][half * NQ:(half + 1) * NQ].T).astype(bf)
        in_maps.append({
            "xq_t": xq_t,
            "xc_t": xc_t,
            "wall1": wall1, "wall2": wall2,
            "maskbias": mb_c,
            "nullv_ext": nve,
        })
    return in_maps, nkt


def assemble_out(results):
    out = np.zeros((B, G, 2 * NQ, D), np.float32)
    for c in range(8):
        b, g, half = c // 4, (c // 2) % 2, c % 2
        out[b, g, half * NQ:(half + 1) * NQ] = results[c]["out"]
    return out


def kernel(**inputs):
    from concourse.bass_utils import run_bass_kernel_spmd

    in_maps, nkt = make_in_maps(**inputs)
    nc = get_nc(nkt)
    res = run_bass_kernel_spmd(nc, in_maps, core_ids=list(range(8)))
    return assemble_out(res.results)


# revision 5
# speedup vs baseline: 1.1558x; 1.1558x over previous
"""Trainium2 Bass kernel for grouped multi-head attention (nn_Attention_8263517077742).

Reference computation (per batch b, group g, with x [2048, 512]):
  xn   = x / max(||x||_2, eps) * sqrt(512)        (rmsnorm over feature dim)
  q    = (xn * gamma_q) @ wq[g].T                 -> 8 heads of 64
  k,v  = (xn * gamma_c) @ wkv[g].T                -> 8 heads of 64
  null k/v prepended along key sequence; scores masked by mask[b]; softmax;
  merged heads projected by wout[g].

Sharding: 8 cores = 4 (b,g) instances x 2 query-sequence halves. Each core
computes attention for its 1024 queries over the shared key set, so output
slices are disjoint and no cross-core communication is needed.

Key ideas on top of the v1 kernel:
  - masked keys contribute exp(-1e30) = 0 to softmax and AV, so the key set
    is compacted host-side to just the unmasked rows (~half of 2048), padded
    with zero rows + -1e30 bias to a fixed NK = NKT*128. Queries (all rows)
    and keys (unmasked rows) become separate input tensors; the rotation
    trick of v1 is no longer needed.
  - all matmul operands are bf16 (psum accumulation stays fp32): same PE rate
    as float32r but native full-rate on HW, half the SBUF traffic, and 2x DVE
    on pure-bf16 copies. fp32 is kept for the norm math and exp bias.
  - output projection contracts over 128 rows (head pairs merged in one
    partition tile) instead of 64, halving its matmul count.
  - psum->sbuf copies of the x transposes go to DVE, not the scalar engine:
    the Activation engine is the exp bottleneck.
  - scores are computed transposed (sT [nk, nq]) so softmax needs no
    transposes: exp on ScalarE with the key mask as a per-partition bias,
    denominators via an extra ones-column per head in v, normalization via
    reciprocal + gpsimd partition-broadcast + multiply.
  - no max-subtraction in softmax (scores are O(10); fp32 exp cannot overflow)
  - null k/v handled as a rank-1 update closing each AV accumulation.
"""

import sys
from contextlib import ExitStack

import numpy as np
import ml_dtypes

if "/opt/trn_rl_repo" not in sys.path:
    sys.path.insert(0, "/opt/trn_rl_repo")

import concourse.bass as bass  # noqa: E402
import concourse.mybir as mybir  # noqa: E402
from concourse import bacc  # noqa: E402
from concourse.tile import TileContext  # noqa: E402
from concourse.masks import make_identity  # noqa: E402

P = 128
D = 512           # feature dim
E = 512           # inner dim (8 heads x 64)
NQ = 1024         # queries per core
H = 8
DH = 64
ET = E // P       # 4 e-tiles
DT = D // P       # 4 d-tiles
QT = NQ // P      # 8 query tiles
VEXT = H * (DH + 1)   # 520: per-head v columns + ones column
F32 = mybir.dt.float32
BF16 = mybir.dt.bfloat16

B, G = 2, 2
NKT_DEFAULT = 9   # key tiles: 1152 key slots; ~1024+5sigma needed for a random mask


def build_nc(reps=1, nkt=NKT_DEFAULT):
    NK = nkt * P
    nc = bacc.Bacc(
        trn_type="TRN2",
        target_bir_lowering=False,
        debug=False,
        enable_asserts=False,
        num_devices=8,
    )
    xqt_ext = nc.declare_dram_parameter("xq_t", [D, NQ], BF16, isOutput=False)
    xct_ext = nc.declare_dram_parameter("xc_t", [D, NK], BF16, isOutput=False)
    # packed weight walls (single DMA each):
    #   wall1 cols: per dj-block of 1024: [wq_t rows dj | wk_t rows dj]; then 4x8 nks
    #   wall2 cols: per block of 1024: [wv_t rows dj | wo_t rows j]
    w1_ext = nc.declare_dram_parameter("wall1", [P, DT * 2 * E + ET * H], BF16, isOutput=False)
    w2_ext = nc.declare_dram_parameter("wall2", [P, DT * 2 * E], BF16, isOutput=False)
    mb_ext = nc.declare_dram_parameter("maskbias", [P, nkt], F32, isOutput=False)
    nve_ext = nc.declare_dram_parameter("nullv_ext", [1, VEXT], BF16, isOutput=False)
    out_ext = nc.declare_dram_parameter("out", [NQ, D], F32, isOutput=True)

    def fchunks(total, step=512):
        c = []
        o = 0
        while o < total:
            c.append((o, min(step, total - o)))
            o += step
        return c

    with TileContext(nc) as tc, ExitStack() as ctx:
        if reps > 1:
            ctx.enter_context(tc.For_i(
                0, reps, 1,
                staggered_reset=True,
                hint_engines=(
                    mybir.EngineType.PE, mybir.EngineType.DVE,
                    mybir.EngineType.Activation, mybir.EngineType.SP,
                    mybir.EngineType.Pool,
                ),
            ))
        # ---- pools that live through prologue + main loop ----
        persist = ctx.enter_context(tc.tile_pool(name="persist", bufs=1))
        kT = [persist.tile([P, NK], BF16, name=f"kT{j}", tag=f"kT{j}") for j in range(ET)]
        qT = [persist.tile([P, NQ], BF16, name=f"qT{j}", tag=f"qT{j}") for j in range(ET)]
        v_ext = [persist.tile([P, VEXT], BF16, name=f"vx{i}", tag=f"vx{i}") for i in range(nkt)]
        # merged heads, pairs per partition tile: rows of mg{j} are
        # e = 128*j .. 128*j+127  (head 2j in rows 0..63, head 2j+1 in 64..127)
        mergedT = [persist.tile([P, NQ], BF16, name=f"mg{j}", tag=f"mg{j}") for j in range(ET)]
        mb_sb = persist.tile([P, nkt], F32, name="mb", tag="mb")
        nve_sb = persist.tile([1, VEXT], BF16, name="nve", tag="nve")

        with tc.tile_pool(name="prolog", bufs=1) as prolog, \
             tc.tile_pool(name="ppsum", bufs=2, space="PSUM") as ppsum:
            xnTq = [prolog.tile([P, NQ], BF16, name=f"xnTq{j}", tag=f"xnTq{j}") for j in range(DT)]
            xnTc = [prolog.tile([P, NK], BF16, name=f"xnTc{j}", tag=f"xnTc{j}") for j in range(DT)]
            onesc = prolog.tile([P, H], BF16, name="onesc", tag="onesc")
            nc.vector.memset(onesc[:, :], 1.0)
            # dummy ops: pull the sqrt/exp table-set loads (~1.3us each) off
            # the first norm tile's critical path
            warm = prolog.tile([1, 2], F32, name="warm", tag="warm")
            nc.vector.memset(warm[:, :], 1.0)
            # pull the sqrt table-set load (~1.3us) off the first norm's
            # critical path; the exp set is loaded after the last sqrt (below)
            # so each set loads exactly once
            nc.scalar.activation(warm[0:1, 0:1], warm[0:1, 0:1],
                                 mybir.ActivationFunctionType.Sqrt)
            wall1 = prolog.tile([P, DT * 2 * E + ET * H], BF16, name="wall1", tag="wall1")
            wall2 = prolog.tile([P, DT * 2 * E], BF16, name="wall2", tag="wall2")

            def wq_s(dj, lo, hi):
                return wall1[:, dj * 1024 + lo:dj * 1024 + hi]

            def wk_s(dj, lo, hi):
                return wall1[:, dj * 1024 + E + lo:dj * 1024 + E + hi]

            def nkc_s(j):
                # null-key column for kT[j] (the fixed last key slot NK-1)
                return wall1[:, DT * 2 * E + j:DT * 2 * E + j + 1]

            def wv_s(dj):
                return wall2[:, dj * 1024:dj * 1024 + E]

            def wo_s(j):
                return wall2[:, j * 1024 + E:j * 1024 + 2 * E]
            # per-column rmsnorm scales (alpha = 1/||x||; sqrt(D)*gamma folded
            # into the weights host-side) broadcast across partitions, plus
            # per-partition alpha columns for the v projection
            acbc = prolog.tile([P, NK], F32, name="acbc", tag="acbc")
            acol = [prolog.tile([P, 1], F32, name=f"acol{t}", tag=f"acol{t}")
                    for t in range(nkt)]
            po1_sb = [prolog.tile([P, D], F32, name=f"po1_{cq}", tag=f"po1_{cq}")
                      for cq in range(QT)]

            # raw x arrives pre-transposed (bf16) from the host; the context
            # side comes first (its norm + k projection gate the first scores)
            for dj in range(DT):
                nc.sync.dma_start(out=xnTc[dj][:, :], in_=xct_ext[dj * P:(dj + 1) * P, :])
            for dj in range(DT):
                nc.sync.dma_start(out=xnTq[dj][:, :], in_=xqt_ext[dj * P:(dj + 1) * P, :])
            nc.sync.dma_start(out=wall1[:, :], in_=w1_ext[:, :])
            nc.sync.dma_start(out=wall2[:, :], in_=w2_ext[:, :])
            nc.sync.dma_start(out=mb_sb[:, :], in_=mb_ext[:, :])
            nc.sync.dma_start(out=nve_sb[:, :], in_=nve_ext[:, :])

            # -- prologue-only pools: norms (column sums of x^2 via matmul) --
            with tc.tile_pool(name="sqpool", bufs=1) as sqpool, \
                 tc.tile_pool(name="npsum", bufs=1, space="PSUM") as npsum, \
                 tc.tile_pool(name="tpsum", bufs=1, space="PSUM") as tpsum:
                ident = sqpool.tile([P, P], F32, name="ident", tag="ident")
                make_identity(nc, ident[:, :])
                ones1 = sqpool.tile([P, 1], BF16, name="ones1", tag="ones1")
                nc.vector.memset(ones1[:, :], 1.0)
                aqbc = sqpool.tile([P, NQ], F32, name="aqbc", tag="aqbc")

                def emit_alpha(xT, n, abc_tile, pfx):
                    xsq = [sqpool.tile([P, n], BF16, name=f"xsq{pfx}{dj}", tag=f"xsq{pfx}{dj}")
                           for dj in range(DT)]
                    for dj in range(DT):
                        nc.vector.tensor_mul(xsq[dj][:, 0:n], xT[dj][:, :], xT[dj][:, :])
                    ps = npsum.tile([1, NK], F32, name=f"ps{pfx}", tag="ps")
                    for (o, w) in fchunks(n):
                        for dj in range(DT):
                            nc.tensor.matmul(
                                ps[:, o:o + w],
                                lhsT=ones1[:, :],
                                rhs=xsq[dj][:, o:o + w],
                                start=(dj == 0), stop=(dj == DT - 1),
                            )
                    nrm = sqpool.tile([1, NK], F32, name="nrm", tag="nrm")
                    nc.scalar.activation(nrm[0:1, 0:n], ps[0:1, 0:n],
                                         mybir.ActivationFunctionType.Sqrt)
                    nc.vector.tensor_scalar_max(nrm[0:1, 0:n], nrm[0:1, 0:n], 1e-12)
                    arow = sqpool.tile([1, NK], F32, name="arow", tag="arow")
                    nc.vector.reciprocal_approx_fast(arow[0:1, 0:n], nrm[0:1, 0:n])
                    nc.gpsimd.partition_broadcast(abc_tile[:, 0:n], arow[0:1, 0:n])

                emit_alpha(xnTc, NK, acbc, "c")
                emit_alpha(xnTq, NQ, aqbc, "q")
                nc.scalar.activation(warm[0:1, 1:2], warm[0:1, 1:2],
                                     mybir.ActivationFunctionType.Exp)
                # alpha columns for the v projection: transpose a 128-col block
                # of the broadcast (every column of the transpose equals the
                # per-token alpha column for that key tile)
                for t in range(nkt):
                    tp = tpsum.tile([P, P], F32, name="tp", tag="tp")
                    nc.tensor.transpose(tp[:, :], acbc[:, t * P:(t + 1) * P], ident[:, :])
                    nc.vector.tensor_copy(acol[t][:, :], tp[:, 0:1])

                # q^T projection (alpha_q folded in on the psum->sbuf copy)
                if True:
                    for j in range(ET):
                        for (o, w) in fchunks(NQ):
                            pq = ppsum.tile([P, 512], F32, name="pk", tag="pk")
                            for dj in range(DT):
                                nc.tensor.matmul(
                                    pq[:, 0:w],
                                    lhsT=wq_s(dj, j * P, (j + 1) * P),
                                    rhs=xnTq[dj][:, o:o + w],
                                    start=(dj == 0), stop=(dj == DT - 1),
                                )
                            nc.vector.tensor_mul(qT[j][:, o:o + w], pq[:, 0:w],
                                                 aqbc[:, o:o + w])


            # -- helpers emitted just-in-time inside the attention loop --
            def emit_vproj(i):
                pv = ppsum.tile([P, 512], F32, name="pk", tag="pk")
                for dj in range(DT):
                    nc.tensor.matmul(
                        pv[:, :],
                        lhsT=xnTc[dj][:, i * P:(i + 1) * P],
                        rhs=wv_s(dj),
                        start=(dj == 0), stop=(dj == DT - 1),
                    )
                src = pv[:, :].rearrange("p (a d) -> p a d", a=H)
                dst = v_ext[i][:, :].rearrange("p (a r) -> p a r", a=H)
                nc.vector.tensor_scalar_mul(dst[:, :, 0:DH], src[:, :, :], acol[i][:, :])
                nc.vector.tensor_copy(dst[:, :, DH:DH + 1],
                                      onesc[:, :].rearrange("p (a r) -> p a r", a=H))
                if i == nkt - 1:
                    # fixed null-kv slot NK-32 (partition starts must be
                    # 32-aligned): overwrite that padding row with the null v
                    # (+ its ones column), bypassing alpha
                    nc.vector.tensor_copy(v_ext[i][P - 32:P - 31, :], nve_sb[0:1, :])

            def emit_kproj(j):
                for (o, w) in fchunks(NK):
                    pk = ppsum.tile([P, 512], F32, name="pk", tag="pk")
                    for dj in range(DT):
                        nc.tensor.matmul(
                            pk[:, 0:w],
                            lhsT=wk_s(dj, j * P, (j + 1) * P),
                            rhs=xnTc[dj][:, o:o + w],
                            start=(dj == 0), stop=(dj == DT - 1),
                        )
                    nc.vector.tensor_mul(kT[j][:, o:o + w], pk[:, 0:w], acbc[:, o:o + w])
                nc.vector.tensor_copy(kT[j][:, NK - 32:NK - 31], nkc_s(j))

            # ---- main attention loop (v/k projections interleaved) ----
            with tc.tile_pool(name="sps", bufs=2, space="PSUM") as sps, \
                 tc.tile_pool(name="avps", bufs=1, space="PSUM") as avps, \
                 tc.tile_pool(name="ppool", bufs=3) as ppool, \
                 tc.tile_pool(name="opool", bufs=3) as opool, \
                 tc.tile_pool(name="rpool", bufs=2) as rpool:

                def emit_oproj_half1(cq):
                    # first half of the output projection (head pairs 0 and 1),
                    # staged to SBUF; interleaved into the act-paced head loop
                    pp = ppsum.tile([P, 512], F32, name="pk", tag="pk")
                    for j in range(2):
                        nc.tensor.matmul(
                            pp[:, :],
                            lhsT=mergedT[j][:, cq * P:(cq + 1) * P],
                            rhs=wo_s(j),
                            start=(j == 0), stop=(j == 1),
                        )
                    nc.vector.tensor_copy(po1_sb[cq][:, :], pp[:, :])

                def emit_oproj_q3(cq):
                    # third quarter (head pair 2), accumulated into the staged
                    # SBUF partials during head 7
                    pp = ppsum.tile([P, 512], F32, name="pk", tag="pk")
                    nc.tensor.matmul(
                        pp[:, :],
                        lhsT=mergedT[2][:, cq * P:(cq + 1) * P],
                        rhs=wo_s(2),
                        start=True, stop=True,
                    )
                    nc.vector.tensor_add(po1_sb[cq][:, :], pp[:, :], po1_sb[cq][:, :])

                emit_vproj(0)
                emit_vproj(1)
                emit_kproj(0)
                for h in range(H):
                    j, off = h // 2, DH * (h % 2)
                    av = avps.tile([DH + 1, NQ], F32, name="av", tag="av")
                    # rows 0..63 = v part, row 64 = softmax denominators r
                    for t in range(nkt):
                        if h == 0 and t + 2 < nkt:
                            emit_vproj(t + 2)
                        # k projections and the first output-projection half are
                        # emitted mid-head, a few tiles after the head's first
                        # scores, so they fill PE idle in the act-paced pipeline
                        # without stalling the next exp behind cross-engine deps
                        if t == 3 and h in (1, 3, 5):
                            emit_kproj((h + 1) // 2)
                        if t in (5, 7) and 4 <= h <= 7:
                            emit_oproj_half1(2 * (h - 4) + (t == 7))
                        st = sps.tile([P, NQ], F32, name="st", tag="st")
                        for (o, w) in fchunks(NQ):
                            nc.tensor.matmul(
                                st[:, o:o + w],
                                lhsT=kT[j][off:off + DH, t * P:(t + 1) * P],
                                rhs=qT[j][off:off + DH, o:o + w],
                                start=True, stop=True,
                            )
                        pt = ppool.tile([P, NQ], BF16, name="pt", tag="pt")
                        nc.scalar.activation(
                            pt[:, :], st[:, :], mybir.ActivationFunctionType.Exp,
                            bias=mb_sb[:, t:t + 1], scale=1.0,
                        )
                        for (o, w) in fchunks(NQ):
                            nc.tensor.matmul(
                                av[:, o:o + w],
                                lhsT=v_ext[t][:, h * 65:h * 65 + 65],
                                rhs=pt[:, o:o + w],
                                start=(t == 0), stop=(t == nkt - 1),
                            )
                    # stage av out of PSUM so the next head can reuse the bank;
                    # the last head has no successor, so it skips the copy and
                    # normalizes straight from PSUM (shorter path to final proj)
                    if h < H - 1:
                        avc = rpool.tile([DH + 1, NQ], F32, name="avc", tag="avc")
                        nc.vector.tensor_copy(avc[:, :], av[:, :])
                    else:
                        avc = av
                    # the approx-recip custom op misreads PSUM and nonzero
                    # base-partition inputs: stage the denominator row through
                    # a base-0 SBUF tile first. The last head normalizes in
                    # column halves so the output projection's final quarter
                    # can start on the first half ~2us earlier.
                    for (o, w) in (fchunks(NQ) if h == H - 1 else [(0, NQ)]):
                        denc = rpool.tile([1, NQ], F32, name="denc", tag="denc")
                        nc.vector.tensor_copy(denc[0:1, 0:w], avc[DH:DH + 1, o:o + w])
                        recip = rpool.tile([1, NQ], F32, name="recip", tag="recip", bufs=1)
                        nc.vector.reciprocal_approx_fast(recip[0:1, 0:w], denc[0:1, 0:w])
                        rbc = rpool.tile([DH, NQ], F32, name="rbc", tag="rbc")
                        nc.gpsimd.partition_broadcast(rbc[:, 0:w], recip[0:1, 0:w])
                        nc.vector.tensor_mul(mergedT[j][off:off + DH, o:o + w],
                                             avc[0:DH, o:o + w], rbc[:, 0:w])

                # ---- output projection tail ----
                # head-pair 2 first: it only needs mergedT[2] (done at head 5),
                # so these matmuls run during head 7's normalize chain and keep
                # the PE p-state warm through the would-be idle window
                for cq in range(QT):
                    emit_oproj_q3(cq)
                # final quarter (head pair 3, gated on head 7) + store
                for cq in range(QT):
                    pp = ppsum.tile([P, 512], F32, name="pk", tag="pk")
                    nc.tensor.matmul(
                        pp[:, :],
                        lhsT=mergedT[3][:, cq * P:(cq + 1) * P],
                        rhs=wo_s(3),
                        start=True, stop=True,
                    )
                    osb = opool.tile([P, D], F32, name="osb", tag="osb")
                    nc.vector.tensor_add(osb[:, :], pp[:, :], po1_sb[cq][:, :])
                    nc.sync.dma_start(out=out_ext[cq * P:(cq + 1) * P, :], in_=osb[:, :])

    nc.compile()
    return nc


_NC_CACHE = {}


def get_nc(nkt=NKT_DEFAULT):
    if nkt not in _NC_CACHE:
        _NC_CACHE[nkt] = build_nc(nkt=nkt)
    return _NC_CACHE[nkt]


def make_in_maps(x, mask, gamma_q, gamma_c, wq, wkv, wout, null_kv, nkt=None):
    x = np.asarray(x, dtype=np.float32)
    mask = np.asarray(mask)
    gamma_q = np.asarray(gamma_q, dtype=np.float32)
    gamma_c = np.asarray(gamma_c, dtype=np.float32)
    wq = np.asarray(wq, dtype=np.float32)
    wkv = np.asarray(wkv, dtype=np.float32)
    wout = np.asarray(wout, dtype=np.float32)
    null_kv = np.asarray(null_kv, dtype=np.float32)

    sqD = np.float32(np.sqrt(D))
    scale = np.float32(DH ** -0.5)
    DI = E
    bf = ml_dtypes.bfloat16

    if nkt is None:
        counts = [int(mask[b].sum()) for b in range(B)]
        # the null key lives at slot NK-32 (32-aligned partition), so the
        # compacted keys must fit below it
        nkt = max(NKT_DEFAULT, -(-(max(counts) + 32) // P))
    NK = nkt * P

    per_g = {}
    for g in range(G):
        wq_t = np.ascontiguousarray((wq[g] * (gamma_q[g] * sqD * scale)[None, :]).T).astype(bf)
        wk_t = np.ascontiguousarray((wkv[g][:DI] * (gamma_c[g] * sqD)[None, :]).T).astype(bf)
        wv_t = np.ascontiguousarray((wkv[g][DI:] * (gamma_c[g] * sqD)[None, :]).T).astype(bf)
        wo_t = np.ascontiguousarray(wout[g].T).astype(bf)
        nullk = null_kv[0, g, :, 0, :]            # [H, DH]
        nks = np.zeros((E, H), np.float32)
        for h in range(H):
            nks[h * DH:(h + 1) * DH, h] = nullk[h]
        nve = np.zeros((H, VEXT), np.float32)
        for h in range(H):
            nve[h, h * 65:h * 65 + 64] = null_kv[1, g, h, 0, :]
            nve[h, h * 65 + 64] = 1.0
        nullk_full = nullk.reshape(E)  # e = h*64+dh
        # packed weight walls (see build_nc): per dj block of 1024 cols,
        # wall1 = [wq | wk] + null-k columns tail, wall2 = [wv | wo]
        wall1 = np.zeros((P, DT * 2 * E + ET * H), bf)
        wall2 = np.zeros((P, DT * 2 * E), bf)
        for dj in range(DT):
            rows = slice(dj * P, (dj + 1) * P)
            wall1[:, dj * 1024:dj * 1024 + E] = wq_t[rows]
            wall1[:, dj * 1024 + E:dj * 1024 + 2 * E] = wk_t[rows]
            wall1[:, DT * 2 * E + dj] = nullk_full[rows].astype(bf)
            wall2[:, dj * 1024:dj * 1024 + E] = wv_t[rows]
            wall2[:, dj * 1024 + E:dj * 1024 + 2 * E] = wo_t[rows]
        per_g[g] = (wall1, wall2, nve.astype(bf)[None, :].sum(axis=1))

    per_b = {}
    for b in range(B):
        idx = np.flatnonzero(mask[b])
        cnt = len(idx)
        mb = np.full(NK, np.float32(-1e30), np.float32)
        mb[:cnt] = 0.0
        mb[NK - 32] = 0.0  # null-kv slot
        per_b[b] = (idx, np.ascontiguousarray(mb.reshape(nkt, P).T))

    in_maps = []
    for c in range(8):
        b, g, half = c // 4, (c // 2) % 2, c % 2
        wall1, wall2, nve = per_g[g]
        idx, mb_c = per_b[b]
        xc_t = np.zeros((D, NK), bf)
        xc_t[:, :len(idx)] = x[b, g][idx].T.astype(bf)
        xq_t = np.ascontiguousarray(x[b, g][half * NQ:(half + 1) * NQ].T).astype(bf)
        in_maps.append({
            "xq_t": xq_t,
            "xc_t": xc_t,
            "wall1": wall1, "wall2": wall2,
            "maskbias": mb_c,
            "nullv_ext": nve,
        })
    return in_maps, nkt


def assemble_out(results):
    out = np.zeros((B, G, 2 * NQ, D), np.float32)
    for c in range(8):
        b, g, half = c // 4, (c // 2) % 2, c % 2
        out[b, g, half * NQ:(half + 1) * NQ] = results[c]["out"]
    return out


def kernel(**inputs):
    from concourse.bass_utils import run_bass_kernel_spmd

    in_maps, nkt = make_in_maps(**inputs)
    nc = get_nc(nkt)
    res = run_bass_kernel_spmd(nc, in_maps, core_ids=list(range(8)))
    return assemble_out(res.results)
